# revision 23
# baseline (speedup 1.0000x reference)
"""Bahdanau-attention kernel for Trainium2 (8 NeuronCores, SPMD data parallel).

Math: the reference's per-step softmax is over a singleton axis, so the
attention weights are exactly 1.0. Hence:
    context  = values.sum(axis=1)            [B, DV]
    attn     = ones(B, T, 1)
    coverage[b, t, 0] = t                    [B, T, 1]
The W1/W2/W3/V MLP cancels out of every output.

Device work: per core, reduce a [B/8, T, DV] shard of `values` over T.
All chunks stream in via plain HWDGE loads; the fp32 adds are spread over
three engines so none exceeds the HBM/DMA roofline:
  - DVE: most chunks, serial tensor_add chain into dacc (plus merging gacc)
  - GpSimd: a few chunks, its own small add chain into gacc
  - PE: a few chunks matmul'd directly vs a ones column into the PSUM
    accumulation group, which also contracts dacc over partitions.
attn/coverage come from a tiny host const tensor, written out by DMA.
"""

import os
import numpy as np

B, T, DV = 32, 2048, 1024
NCORES = 8
BP = B // NCORES          # 4 batches per core
TCH = 128                 # t-chunk rows = SBUF partitions
NCH = T // TCH            # 16 chunks of [128, DV] per batch
NSPLIT = 512              # PSUM bank free-dim limit (f32)
NJ = DV // NSPLIT         # 2 psum column groups

# per-batch engine assignment: early batches lean on DVE/gpsimd chains
# (they have the whole kernel to finish); the last-loaded batch goes all-PE
# so its reduction completes within ~1.3us of each chunk's arrival and the
# kernel tail stays short.
PE_CHUNKS = {
    0: (5, 10, 15),
    1: (5, 10, 15),
    2: (3, 6, 9, 12, 15),
    3: tuple(range(6, 16)),
}
GP_CHUNKS = {
    0: (2, 7, 12),
    1: (2, 7, 12),
    2: (2, 7, 11),
    3: (),
}
# batches whose DVE chunks are reduced as independent pair-adds (low
# latency, no serial chain) with the pair sums contracted on PE
PAIR_BATCHES = (3,)

_CACHE = {}
LAST = {}                 # exec_time_ns etc. for the test harness


def _build_nc():
    import concourse.tile as tile
    from concourse import bacc, mybir
    from contextlib import ExitStack

    f32 = mybir.dt.float32
    nc = bacc.Bacc(
        "TRN2", target_bir_lowering=False, debug=False, num_devices=NCORES
    )

    vals = nc.dram_tensor("vals", [BP, T, DV], f32, kind="ExternalInput").ap()
    consts = nc.dram_tensor("consts", [2, T], f32, kind="ExternalInput").ap()
    ctx_out = nc.dram_tensor("ctx_out", [BP, DV], f32, kind="ExternalOutput").ap()
    attn_out = nc.dram_tensor("attn_out", [BP, T, 1], f32, kind="ExternalOutput").ap()
    cov_out = nc.dram_tensor("cov_out", [BP, T, 1], f32, kind="ExternalOutput").ap()

    with tile.TileContext(nc) as tc, ExitStack() as ctx:
        cpool = ctx.enter_context(tc.tile_pool(name="const", bufs=1))
        vpool = ctx.enter_context(tc.tile_pool(name="vals", bufs=28))
        dpool = ctx.enter_context(tc.tile_pool(name="dacc", bufs=1))
        ppool = ctx.enter_context(tc.tile_pool(name="ps", bufs=1, space="PSUM"))
        opool = ctx.enter_context(tc.tile_pool(name="out", bufs=2))

        ones_t = cpool.tile([128, 1], f32)
        nc.vector.memset(ones_t[:], 1.0)

        const_t = cpool.tile([2, T], f32)
        nc.sync.dma_start(out=const_t[:], in_=consts[:])

        # attn/coverage writes go on the scalar HWDGE ring, issued as soon
        # as const_t lands, so the sync ring carries only the big loads.
        for b in range(BP):
            nc.scalar.dma_start(out=attn_out[b:b + 1, :, 0], in_=const_t[0:1, :])
            nc.scalar.dma_start(out=cov_out[b:b + 1, :, 0], in_=const_t[1:2, :])

        for b in range(BP):
            pe_ch = PE_CHUNKS[b]
            gp_ch = GP_CHUNKS[b]
            pair_mode = b in PAIR_BATCHES
            n_dv = NCH - len(pe_ch) - len(gp_ch)
            dacc = gacc = None
            pairs = []
            if n_dv and not pair_mode:
                dacc = dpool.tile(
                    [TCH, DV], f32, name=f"dacc{b}", tag=f"dacc{b}")
            if gp_ch:
                gacc = dpool.tile(
                    [TCH, DV], f32, name=f"gacc{b}", tag=f"gacc{b}")
            pe_tiles = {}
            ndv = ngp = 0
            dfirst = gfirst = None
            for k in range(NCH):
                vt = vpool.tile([TCH, DV], f32, name=f"vt{b}_{k}", tag="vt")
                nc.sync.dma_start(
                    out=vt[:], in_=vals[b, k * TCH:(k + 1) * TCH, :])
                if k in pe_ch:
                    pe_tiles[k] = vt
                elif k in gp_ch:
                    ngp += 1
                    if ngp == 1:
                        gfirst = vt
                    elif ngp == 2:
                        nc.gpsimd.tensor_add(gacc[:], gfirst[:], vt[:])
                    else:
                        nc.gpsimd.tensor_add(gacc[:], gacc[:], vt[:])
                elif pair_mode:
                    ndv += 1
                    if ndv % 2 == 1:
                        dfirst = vt
                    else:
                        pr = dpool.tile(
                            [TCH, DV], f32,
                            name=f"pr{b}_{ndv // 2}", tag=f"pr{b}_{ndv // 2}")
                        nc.vector.tensor_add(pr[:], dfirst[:], vt[:])
                        pairs.append(pr)
                else:
                    ndv += 1
                    if ndv == 1:
                        dfirst = vt
                    elif ndv == 2:
                        nc.vector.tensor_add(dacc[:], dfirst[:], vt[:])
                    else:
                        nc.vector.tensor_add(dacc[:], dacc[:], vt[:])
            if gacc is not None:
                # fold the gpsimd partial into dacc (DVE)
                nc.vector.tensor_add(dacc[:], dacc[:], gacc[:])

            ps = [
                ppool.tile([1, NSPLIT], f32, name=f"ps{b}_{j}", tag=f"ps{b}_{j}")
                for j in range(NJ)
            ]
            # group order: pair sums first (ready early), then direct chunks
            # in arrival order; dacc contraction second-to-last so the stop
            # matmul is a late direct chunk
            order = pairs + [pe_tiles[k] for k in pe_ch]
            if dacc is not None:
                order = order[:-1] + [dacc] + order[-1:]
            for i, src in enumerate(order):
                for j in range(NJ):
                    sl = slice(j * NSPLIT, (j + 1) * NSPLIT)
                    nc.tensor.matmul(
                        ps[j][:], ones_t[:], src[:, sl],
                        start=(i == 0), stop=(i == len(order) - 1))

            ot = opool.tile([1, DV], f32, name=f"ot{b}", tag="ot")
            for j in range(NJ):
                nc.scalar.copy(ot[:, j * NSPLIT:(j + 1) * NSPLIT], ps[j][:])
            # context write on the scalar ring: keeps the sync HWDGE FIFO
            # free of output DMAs that would head-of-line block later loads
            nc.scalar.dma_start(out=ctx_out[b:b + 1, :], in_=ot[0:1, :])

    nc.compile()
    return nc


def kernel(query=None, values=None, **unused_weights):
    from concourse.bass_utils import run_bass_kernel_spmd

    values = np.ascontiguousarray(np.asarray(values, dtype=np.float32))
    assert values.shape == (B, T, DV), values.shape

    if "nc" not in _CACHE:
        _CACHE["nc"] = _build_nc()
    nc = _CACHE["nc"]

    consts = np.stack(
        [np.ones(T, dtype=np.float32), np.arange(T, dtype=np.float32)]
    )
    core_ids = list(range(NCORES))
    in_maps = [
        {"vals": values[c * BP:(c + 1) * BP], "consts": consts}
        for c in core_ids
    ]

    trace = bool(int(os.environ.get("BASS_KERNEL_TRACE", "0")))
    res = run_bass_kernel_spmd(nc, in_maps, core_ids, trace=trace)
    LAST["exec_time_ns"] = res.exec_time_ns
    LAST["results"] = res

    context = np.concatenate([res.results[c]["ctx_out"] for c in core_ids], axis=0)
    attn = np.concatenate([res.results[c]["attn_out"] for c in core_ids], axis=0)
    coverage = np.concatenate([res.results[c]["cov_out"] for c in core_ids], axis=0)
    return context, attn, coverage


# revision 26
# speedup vs baseline: 1.1289x; 1.1289x over previous
"""Bahdanau-attention kernel for Trainium2 (8 NeuronCores, SPMD data parallel).

Math: the reference's per-step softmax is over a singleton axis, so the
attention weights are exactly 1.0. Hence:
    context  = values.sum(axis=1)            [B, DV]
    attn     = ones(B, T, 1)
    coverage[b, t, 0] = t                    [B, T, 1]
The W1/W2/W3/V MLP cancels out of every output.

Device work: per core, reduce a [B/8, T, DV] shard of `values` over T.
All chunks stream in via plain HWDGE loads; the fp32 adds are spread over
three engines so none exceeds the HBM/DMA roofline:
  - DVE: most chunks, serial tensor_add chain into dacc (plus merging gacc)
  - GpSimd: a few chunks, its own small add chain into gacc
  - PE: a few chunks matmul'd directly vs a ones column into the PSUM
    accumulation group, which also contracts dacc over partitions.
attn/coverage come from a tiny host const tensor, written out by DMA.
"""

import os
import numpy as np

B, T, DV = 32, 2048, 1024
NCORES = 8
BP = B // NCORES          # 4 batches per core
TCH = 128                 # t-chunk rows = SBUF partitions
NCH = T // TCH            # 16 chunks of [128, DV] per batch
NSPLIT = 512              # PSUM bank free-dim limit (f32)
NJ = DV // NSPLIT         # 2 psum column groups

# Engine roles (measured per-chunk costs: DVE add 1.22us, gpsimd add
# 2.4us, PE direct contraction ~2.5us = 4 MM insts):
#   - gpsimd: 4 chunks per batch in its own chain (gacc)
#   - DVE: everything else, as TWO half-chains per batch (dacc_a: k<8,
#     dacc_b: k>=8) so the b's final partial is ready ~1 add after its
#     last chunk lands; gacc folds into dacc_a mid-stream
#   - PE: only the two cross-partition contractions per batch
GP_CHUNKS = (1, 4, 7, 10)
GP_FOLD_AFTER = 13        # emit the gacc->dacc_a fold after this chunk

_CACHE = {}
LAST = {}                 # exec_time_ns etc. for the test harness


def _build_nc():
    import concourse.tile as tile
    from concourse import bacc, mybir
    from contextlib import ExitStack

    f32 = mybir.dt.float32
    nc = bacc.Bacc(
        "TRN2", target_bir_lowering=False, debug=False, num_devices=NCORES
    )

    vals = nc.dram_tensor("vals", [BP, T, DV], f32, kind="ExternalInput").ap()
    consts = nc.dram_tensor("consts", [2, T], f32, kind="ExternalInput").ap()
    ctx_out = nc.dram_tensor("ctx_out", [BP, DV], f32, kind="ExternalOutput").ap()
    attn_out = nc.dram_tensor("attn_out", [BP, T, 1], f32, kind="ExternalOutput").ap()
    cov_out = nc.dram_tensor("cov_out", [BP, T, 1], f32, kind="ExternalOutput").ap()

    with tile.TileContext(nc) as tc, ExitStack() as ctx:
        cpool = ctx.enter_context(tc.tile_pool(name="const", bufs=1))
        vpool = ctx.enter_context(tc.tile_pool(name="vals", bufs=24))
        dpool = ctx.enter_context(tc.tile_pool(name="dacc", bufs=1))
        ppool = ctx.enter_context(tc.tile_pool(name="ps", bufs=1, space="PSUM"))
        opool = ctx.enter_context(tc.tile_pool(name="out", bufs=2))

        ones_t = cpool.tile([128, 1], f32)
        nc.vector.memset(ones_t[:], 1.0)

        const_t = cpool.tile([2, T], f32)
        nc.sync.dma_start(out=const_t[:], in_=consts[:])

        # attn/coverage writes go on the scalar HWDGE ring, issued as soon
        # as const_t lands, so the sync ring carries only the big loads.
        for b in range(BP):
            nc.scalar.dma_start(out=attn_out[b:b + 1, :, 0], in_=const_t[0:1, :])
            nc.scalar.dma_start(out=cov_out[b:b + 1, :, 0], in_=const_t[1:2, :])

        for b in range(BP):
            dacc_a = dpool.tile([TCH, DV], f32, name=f"dacca{b}", tag=f"dacca{b}")
            dacc_b = dpool.tile([TCH, DV], f32, name=f"daccb{b}", tag=f"daccb{b}")
            gacc = dpool.tile([TCH, DV], f32, name=f"gacc{b}", tag=f"gacc{b}")
            na = nb = ngp = 0
            afirst = bfirst = gfirst = None
            for k in range(NCH):
                vt = vpool.tile([TCH, DV], f32, name=f"vt{b}_{k}", tag="vt")
                nc.sync.dma_start(
                    out=vt[:], in_=vals[b, k * TCH:(k + 1) * TCH, :])
                if k in GP_CHUNKS:
                    ngp += 1
                    if ngp == 1:
                        gfirst = vt
                    elif ngp == 2:
                        nc.gpsimd.tensor_add(gacc[:], gfirst[:], vt[:])
                    else:
                        nc.gpsimd.tensor_add(gacc[:], gacc[:], vt[:])
                elif k < NCH // 2:
                    na += 1
                    if na == 1:
                        afirst = vt
                    elif na == 2:
                        nc.vector.tensor_add(dacc_a[:], afirst[:], vt[:])
                    else:
                        nc.vector.tensor_add(dacc_a[:], dacc_a[:], vt[:])
                else:
                    nb += 1
                    if nb == 1:
                        bfirst = vt
                    elif nb == 2:
                        nc.vector.tensor_add(dacc_b[:], bfirst[:], vt[:])
                    else:
                        nc.vector.tensor_add(dacc_b[:], dacc_b[:], vt[:])
                if k == GP_FOLD_AFTER:
                    nc.vector.tensor_add(dacc_a[:], dacc_a[:], gacc[:])

            ps = [
                ppool.tile([1, NSPLIT], f32, name=f"ps{b}_{j}", tag=f"ps{b}_{j}")
                for j in range(NJ)
            ]
            order = [dacc_a, dacc_b]
            for i, src in enumerate(order):
                for j in range(NJ):
                    sl = slice(j * NSPLIT, (j + 1) * NSPLIT)
                    nc.tensor.matmul(
                        ps[j][:], ones_t[:], src[:, sl],
                        start=(i == 0), stop=(i == len(order) - 1))

            ot = opool.tile([1, DV], f32, name=f"ot{b}", tag="ot")
            for j in range(NJ):
                nc.scalar.copy(ot[:, j * NSPLIT:(j + 1) * NSPLIT], ps[j][:])
            # context write on the scalar ring: keeps the sync HWDGE FIFO
            # free of output DMAs that would head-of-line block later loads
            nc.scalar.dma_start(out=ctx_out[b:b + 1, :], in_=ot[0:1, :])

    nc.compile()
    return nc


def kernel(query=None, values=None, **unused_weights):
    from concourse.bass_utils import run_bass_kernel_spmd

    values = np.ascontiguousarray(np.asarray(values, dtype=np.float32))
    assert values.shape == (B, T, DV), values.shape

    if "nc" not in _CACHE:
        _CACHE["nc"] = _build_nc()
    nc = _CACHE["nc"]

    consts = np.stack(
        [np.ones(T, dtype=np.float32), np.arange(T, dtype=np.float32)]
    )
    core_ids = list(range(NCORES))
    in_maps = [
        {"vals": values[c * BP:(c + 1) * BP], "consts": consts}
        for c in core_ids
    ]

    trace = bool(int(os.environ.get("BASS_KERNEL_TRACE", "0")))
    res = run_bass_kernel_spmd(nc, in_maps, core_ids, trace=trace)
    LAST["exec_time_ns"] = res.exec_time_ns
    LAST["results"] = res

    context = np.concatenate([res.results[c]["ctx_out"] for c in core_ids], axis=0)
    attn = np.concatenate([res.results[c]["attn_out"] for c in core_ids], axis=0)
    coverage = np.concatenate([res.results[c]["cov_out"] for c in core_ids], axis=0)
    return context, attn, coverage
